# revision 29
# baseline (speedup 1.0000x reference)
"""CoSSM (bidirectional Mamba-style SSM) Trainium2 Bass kernel.

Sharding over 8 cores: (stream g/r) x (batch 0/1) x (d_inner half 0/1).
Each core, for its (stream, batch):
  - in_proj x (all 1536 ch, channel-permuted so own half = tiles 0..5),
    recomputed per direction, fused with the causal/anticausal depthwise
    conv (diagonal PE matmuls) + native Silu, chunked over time with a
    4-column halo carried between chunks (no DRAM round trip for x)
  - xproj (full-d contraction) -> dt/B/C rows; dtproj + native Softplus
  - fwd + bwd selective scans (16 states, DVE tensor_tensor_scan);
    bwd consumes u/dl/B/C through time-reversed DMA reads (no flips)
  - D residual, bidirectional average (0.5 folded into out_proj w),
    silu(z) gate, partial out_proj -> (2048, 768)
Host sums the two partial outputs per (stream, batch).

Weights are SBUF-resident (loaded once, bf16); activations bf16 where
cheap, f32 on the delta/scan-state path.

Self-contained: hardcodes shapes. Inputs use the reference setup_inputs names.
"""
import numpy as np
import ml_dtypes

import concourse.bass as bass
import concourse.bacc as bacc
import concourse.tile as tile
import concourse.mybir as mybir
from concourse.bass_utils import run_bass_kernel_spmd

F32 = mybir.dt.float32
BF16 = mybir.dt.bfloat16
AF = mybir.ActivationFunctionType
OP = mybir.AluOpType

D_MODEL = 768
D_STATE = 16
D_CONV = 4
D_INNER = 1536
DT_RANK = 48
NBATCH = 2
L = 2048
HALF = 768
NT_FULL = 12      # 128-tiles over d_inner
NT_HALF = 6       # 128-tiles over own half
NK = 6            # 128-chunks over d_model contraction
TCH = 512         # time chunk for phase A
NTCH = L // TCH
SEG = 256         # time segment for the scan + epilogue
NSEG = L // SEG
HALO = 4

_PROGRAM_CACHE = {}


def _ap(t, offset, ap):
    return bass.AP(tensor=t.tensor, offset=offset, ap=[list(a) for a in ap])


def _bc6(view2d):
    """[128, T] AP -> [128, 6, T] read view (0-stride middle dim)."""
    a = view2d
    return bass.AP(tensor=a.tensor, offset=a.offset,
                   ap=[list(a.ap[0]), [0, NT_HALF], list(a.ap[1])])


def build_program(a_vals_f, a_vals_b):
    nc = bacc.Bacc("TRN2", target_bir_lowering=False, debug=False, num_devices=8)

    def din(name, shape, dt):
        return nc.dram_tensor(name, list(shape), dt, kind="ExternalInput").ap()

    hid_T = din("hid_T", (D_MODEL, L), BF16)
    w_in_x_T = din("w_in_x_T", (D_MODEL, D_INNER), BF16)
    w_zo_T = din("w_zo_T", (D_MODEL, 2 * D_MODEL), BF16)  # [w_in_z | 0.5*w_out]
    cdiag = {"f": din("cdiag_f", (NT_FULL * D_CONV, 128, 128), BF16),
             "b": din("cdiag_b", (NT_FULL * D_CONV, 128, 128), BF16)}
    cbias = {"f": din("cbias_f", (D_INNER,), F32),
             "b": din("cbias_b", (D_INNER,), F32)}
    w_x_T = {"f": din("w_x_T_f", (D_INNER, 80), BF16),
             "b": din("w_x_T_b", (D_INNER, 80), BF16)}
    w_dt_T = {"f": din("w_dt_T_f", (DT_RANK, HALF), F32),
              "b": din("w_dt_T_b", (DT_RANK, HALF), F32)}
    dt_bias = {"f": din("dt_bias_f", (HALF,), F32),
               "b": din("dt_bias_b", (HALF,), F32)}
    d_res = {"f": din("d_f", (HALF,), F32), "b": din("d_b", (HALF,), F32)}

    out = nc.dram_tensor("out", [L, D_MODEL], F32, kind="ExternalOutput").ap()

    u_sp = {d: nc.dram_tensor(f"u_sp_{d}", [NT_HALF, 128, L], BF16).ap()
            for d in "fb"}
    dl_sp = {d: nc.dram_tensor(f"dl_sp_{d}", [NT_HALF, 128, L], F32).ap()
             for d in "fb"}
    r_sp = {d: nc.dram_tensor(f"r_sp_{d}", [NT_HALF, 128, L], F32).ap()
            for d in "fb"}
    bc_dram = {d: nc.dram_tensor(f"bc_{d}", [2 * D_STATE, L], BF16).ap()
               for d in "fb"}

    a_vals = {"f": a_vals_f, "b": a_vals_b}

    import contextlib
    with tile.TileContext(nc) as tc, contextlib.ExitStack() as ctx:
        WPOOL = ctx.enter_context(tc.tile_pool(name="wsmall", bufs=1))
        AP_ = ctx.enter_context(tc.tile_pool(name="phaseA", bufs=1))
        SP = ctx.enter_context(tc.tile_pool(name="work", bufs=2))
        SC = ctx.enter_context(tc.tile_pool(name="scan1", bufs=1))
        SC2 = ctx.enter_context(tc.tile_pool(name="scan2", bufs=3))
        DP = ctx.enter_context(tc.tile_pool(name="phaseD", bufs=1))
        PS = ctx.enter_context(tc.tile_pool(name="psum", bufs=2, space="PSUM"))
        PS1 = ctx.enter_context(tc.tile_pool(name="psum1", bufs=1, space="PSUM"))
        PER = ctx.enter_context(tc.tile_pool(name="persist", bufs=1))

        # ---- resident weights (one DMA each) ----
        t_wbig = WPOOL.tile([128, NK, D_INNER], BF16, tag="wbig")
        nc.sync.dma_start(
            out=t_wbig,
            in_=_ap(w_in_x_T, 0,
                    [[D_INNER, 128], [128 * D_INNER, NK], [1, D_INNER]]))

        def load_cols(src, n, tagn):
            t = WPOOL.tile([128, n], F32, tag=tagn)
            nc.sync.dma_start(out=t, in_=_ap(src, 0, [[1, 128], [128, n]]))
            return t

        t_cbias = {d: load_cols(cbias[d], NT_FULL, f"cb{d}") for d in "fb"}
        t_dtb = {d: load_cols(dt_bias[d], NT_HALF, f"db{d}") for d in "fb"}
        t_dcol = {d: load_cols(d_res[d], NT_HALF, f"dd{d}") for d in "fb"}
        t_wx = {}
        for d in "fb":
            t = WPOOL.tile([128, NT_FULL, 80], BF16, tag=f"wx{d}")
            nc.sync.dma_start(
                out=t, in_=_ap(w_x_T[d], 0,
                               [[80, 128], [128 * 80, NT_FULL], [1, 80]]))
            t_wx[d] = t
        t_wdt = {}
        for d in "fb":
            t = WPOOL.tile([128, HALF], F32, tag=f"wdt{d}")
            nc.sync.dma_start(out=t[0:DT_RANK, :], in_=w_dt_T[d])
            t_wdt[d] = t

        # conv diag tiles: loaded per direction into one aliased slot
        def load_cdiag(d):
            t = WPOOL.tile([128, NT_FULL * D_CONV, 128], BF16, tag="cdiag")
            nc.sync.dma_start(
                out=t, in_=_ap(cdiag[d], 0,
                               [[128, 128], [128 * 128, NT_FULL * D_CONV],
                                [1, 128]]))
            return t

        t_xdbl = PER.tile([128, L], F32, tag="xdbl")
        t_xw = PER.tile([128, NT_FULL, TCH + HALO], BF16, tag="xw")
        carry = {d: PER.tile([128, D_STATE, NT_HALF], F32, tag=f"carry{d}",
                             name=f"carry_{d}")
                 for d in "fb"}

        # ---- phase A: fused in_proj + conv + silu + xproj + dt ----
        def emit_phaseA_chunk(d, tci, t_cd):
            t0 = tci * TCH
            first = (tci == 0) if d == "f" else (tci == NTCH - 1)
            th = AP_.tile([128, NK, TCH], BF16, tag="hidw", bufs=2)
            nc.sync.dma_start(
                out=th, in_=_ap(hid_T, t0, [[L, 128], [128 * L, NK], [1, TCH]]))
            # halo carry in xw: fwd cols 0:4 <- prev cols 512:516 (or zero);
            # bwd cols 512:516 <- prev cols 0:4 (or zero)
            if d == "f":
                if first:
                    nc.vector.memset(t_xw[:, :, 0:HALO].bitcast(F32), 0.0)
                else:
                    nc.vector.tensor_copy(t_xw[:, :, 0:HALO],
                                          t_xw[:, :, TCH:TCH + HALO])
            else:
                if first:
                    nc.vector.memset(t_xw[:, :, TCH:].bitcast(F32), 0.0)
                else:
                    nc.vector.tensor_copy(t_xw[:, :, TCH:TCH + HALO],
                                          t_xw[:, :, 0:HALO])
            xcol = HALO if d == "f" else 0

            t_useg = AP_.tile([128, NT_HALF, TCH], BF16, tag="useg")
            t_dlseg = AP_.tile([128, NT_HALF, TCH], F32, tag="dlseg")
            px = PS1.tile([128, TCH], F32, tag="xproj")
            for i in range(NT_FULL):
                ip = PS.tile([128, TCH], F32, tag="mm512")
                for k in range(NK):
                    nc.tensor.matmul(ip[:], t_wbig[:, k, i * 128:(i + 1) * 128],
                                     th[:, k, :],
                                     start=(k == 0), stop=(k == NK - 1))
                nc.scalar.copy(t_xw[:, i, xcol:xcol + TCH], ip[:])
                cp = PS.tile([128, TCH], F32, tag="mm512")
                for k in range(D_CONV):
                    off = 1 + k if d == "f" else 3 - k
                    nc.tensor.matmul(cp[:], t_cd[:, i * D_CONV + k, :],
                                     t_xw[:, i, off:off + TCH],
                                     start=(k == 0), stop=(k == D_CONV - 1))
                if i < NT_HALF:
                    u_i = t_useg[:, i, :]
                else:
                    u_hi = SP.tile([128, TCH], BF16, tag="u_hi")
                    u_i = u_hi[:]
                nc.scalar.activation(u_i, cp[:], AF.Silu,
                                     bias=t_cbias[d][:, i:i + 1], scale=1.0)
                nc.tensor.matmul(px[0:80, :], t_wx[d][:, i, :], u_i,
                                 start=(i == 0), stop=(i == NT_FULL - 1))
            nc.sync.dma_start(
                out=_ap(u_sp[d], t0, [[L, 128], [128 * L, NT_HALF], [1, TCH]]),
                in_=t_useg[:])
            nc.scalar.copy(t_xdbl[0:80, t0:t0 + TCH], px[0:80, :])
            for m in range(NT_HALF):
                dp = PS.tile([128, TCH], F32, tag="mm512")
                nc.tensor.matmul(dp[:],
                                 t_wdt[d][0:DT_RANK, m * 128:(m + 1) * 128],
                                 t_xdbl[0:DT_RANK, t0:t0 + TCH],
                                 start=True, stop=True)
                nc.scalar.activation(t_dlseg[:, m, :], dp[:], AF.Exp,
                                     bias=t_dtb[d][:, m:m + 1], scale=1.0)
            # softplus: one batched in-place ln(1 + e^x) pass (avoids
            # per-m exp<->ln activation-table thrash)
            nc.scalar.activation(t_dlseg[:], t_dlseg[:], AF.Ln,
                                 bias=1.0, scale=1.0)
            nc.sync.dma_start(
                out=_ap(dl_sp[d], t0, [[L, 128], [128 * L, NT_HALF], [1, TCH]]),
                in_=t_dlseg[:])

        # ---- scan ----
        def emit_scan_seg(d, s):
            # Scan seg s covers natural times [base, base+SEG). For d == "b"
            # all data stays in natural time order; only the scan instruction
            # operands use reversed views so the recurrence runs backward.
            rev = (d == "b")
            base = (L - (s + 1) * SEG) if rev else s * SEG
            u_seg = SC.tile([128, NT_HALF, SEG], BF16, tag="u_seg", bufs=2)
            nc.sync.dma_start(
                out=u_seg,
                in_=_ap(u_sp[d], base,
                        [[L, 128], [128 * L, NT_HALF], [1, SEG]]))
            dl_seg = SC.tile([128, NT_HALF, SEG], F32, tag="dl_seg")
            nc.sync.dma_start(
                out=dl_seg,
                in_=_ap(dl_sp[d], base,
                        [[L, 128], [128 * L, NT_HALF], [1, SEG]]))
            bc = SC.tile([128, 2 * D_STATE, SEG], BF16, tag="bc")
            nc.sync.dma_start(
                out=bc,
                in_=_ap(bc_dram[d], base,
                        [[0, 128], [L, 2 * D_STATE], [1, SEG]]))
            ud_seg = SC.tile([128, NT_HALF, SEG], BF16, tag="ud_seg")
            nc.vector.tensor_tensor(ud_seg[:], dl_seg[:], u_seg[:], OP.mult)
            y_seg = SC.tile([128, NT_HALF, SEG], F32, tag="y_seg")
            for j in range(D_STATE):
                a_j = SC2.tile([128, NT_HALF, SEG], BF16, tag="a_j")
                nc.scalar.activation(a_j[:], dl_seg[:], AF.Exp, bias=0.0,
                                     scale=float(a_vals[d][j]))
                b_j = SC2.tile([128, NT_HALF, SEG], BF16, tag="b_j")
                nc.vector.tensor_tensor(b_j[:], ud_seg[:], _bc6(bc[:, j, :]),
                                        OP.mult)
                h_j = SC2.tile([128, NT_HALF, SEG], BF16, tag="h_j")

                def sop(t, i):
                    v = t[:, i, :]
                    if not rev:
                        return v
                    return bass.AP(tensor=v.tensor, offset=v.offset + SEG - 1,
                                   ap=[list(v.ap[0]), [-1, SEG]])

                for i in range(NT_HALF):
                    init = 0.0 if s == 0 else carry[d][:, j, i:i + 1]
                    nc.vector.tensor_tensor_scan(
                        sop(h_j, i), sop(a_j, i), sop(b_j, i), init,
                        OP.mult, OP.add)
                if s < NSEG - 1:
                    # scan-order last element: natural col SEG-1 (fwd), 0 (bwd)
                    nc.vector.tensor_copy(
                        carry[d][:, j, :],
                        bass.AP(tensor=h_j.tensor,
                                offset=h_j[:].offset +
                                (0 if rev else SEG - 1),
                                ap=[list(h_j[:].ap[0]), [SEG, NT_HALF]]))
                nc.gpsimd.tensor_tensor(h_j[:], h_j[:],
                                        _bc6(bc[:, D_STATE + j, :]), OP.mult)
                if j == 0:
                    nc.gpsimd.tensor_copy(out=y_seg[:], in_=h_j[:])
                else:
                    nc.gpsimd.tensor_tensor(y_seg[:], y_seg[:], h_j[:], OP.add)
            r_seg = SC.tile([128, NT_HALF, SEG], F32, tag="r_seg")
            for i in range(NT_HALF):
                nc.vector.scalar_tensor_tensor(
                    r_seg[:, i, :], u_seg[:, i, :],
                    t_dcol[d][:, i:i + 1], y_seg[:, i, :], OP.mult, OP.add)
            nc.sync.dma_start(
                out=_ap(r_sp[d], base,
                        [[L, 128], [128 * L, NT_HALF], [1, SEG]]),
                in_=r_seg[:])

        # ---- phase D: z-gate + combine + out_proj ----
        def emit_D_seg(s):
            t0 = s * SEG
            th = AP_.tile([128, NK, SEG], BF16, tag="hidw", bufs=2)
            nc.sync.dma_start(
                out=th, in_=_ap(hid_T, t0, [[L, 128], [128 * L, NK], [1, SEG]]))
            gate = DP.tile([128, NT_HALF, SEG], BF16, tag="gate")
            for m in range(NT_HALF):
                zp = PS.tile([128, SEG], F32, tag="mm512")
                for k in range(NK):
                    nc.tensor.matmul(zp[:], t_wbig[:, k, m * 128:(m + 1) * 128],
                                     th[:, k, :],
                                     start=(k == 0), stop=(k == NK - 1))
                nc.scalar.activation(gate[:, m, :], zp[:], AF.Silu, bias=0.0,
                                     scale=1.0)
            rf = DP.tile([128, NT_HALF, SEG], F32, tag="rf")
            nc.sync.dma_start(
                out=rf, in_=_ap(r_sp["f"], t0,
                                [[L, 128], [128 * L, NT_HALF], [1, SEG]]))
            rb = DP.tile([128, NT_HALF, SEG], F32, tag="rb")
            nc.sync.dma_start(
                out=rb, in_=_ap(r_sp["b"], t0,
                                [[L, 128], [128 * L, NT_HALF], [1, SEG]]))
            nc.vector.tensor_tensor(rf[:], rf[:], rb[:], OP.add)
            yg = DP.tile([128, NT_HALF, SEG], BF16, tag="yg")
            nc.vector.tensor_tensor(yg[:], rf[:], gate[:], OP.mult)
            for tcc in range(SEG // 128):
                oseg = SP.tile([128, D_MODEL], F32, tag="oseg")
                for nh in range(2):
                    po = PS.tile([128, 384], F32, tag="oproj")
                    for i in range(NT_HALF):
                        nc.tensor.matmul(
                            po[:], yg[:, i, tcc * 128:(tcc + 1) * 128],
                            t_wbig[:, i, D_MODEL + nh * 384:
                                   D_MODEL + (nh + 1) * 384],
                            start=(i == 0), stop=(i == NT_HALF - 1))
                    nc.scalar.copy(oseg[:, nh * 384:(nh + 1) * 384], po[:])
                nc.sync.dma_start(
                    out=out[t0 + tcc * 128:t0 + (tcc + 1) * 128, :],
                    in_=oseg[:])

        # ---- sequencing: overlap scan(f) with A(b), scan(b) with D ----
        t_cd = load_cdiag("f")
        for tci in range(NTCH):
            emit_phaseA_chunk("f", tci, t_cd)
        nc.gpsimd.dma_start(out=bc_dram["f"],
                            in_=t_xdbl[DT_RANK:DT_RANK + 2 * D_STATE, :])
        t_cd = load_cdiag("b")
        for s in range(NSEG):
            emit_scan_seg("f", s)
            if s % 2 == 1:
                emit_phaseA_chunk("b", NTCH - 1 - s // 2, t_cd)
        nc.gpsimd.dma_start(out=bc_dram["b"],
                            in_=t_xdbl[DT_RANK:DT_RANK + 2 * D_STATE, :])
        # reload wbig slot with [w_in_z | 0.5*w_out] for phase D
        nc.sync.dma_start(
            out=t_wbig,
            in_=_ap(w_zo_T, 0,
                    [[2 * D_MODEL, 128], [128 * 2 * D_MODEL, NK],
                     [1, 2 * D_MODEL]]))
        for s in range(NSEG):
            emit_scan_seg("b", s)
            emit_D_seg(NSEG - 1 - s)

    nc.compile()
    return nc


def _diags(w):  # (1536, 4) -> (48, 128, 128) diag tiles
    o = np.zeros((NT_FULL * D_CONV, 128, 128), np.float32)
    for i in range(NT_FULL):
        for k in range(D_CONV):
            np.fill_diagonal(o[i * D_CONV + k], w[i * 128:(i + 1) * 128, k])
    return o


def _bf(a):
    return np.ascontiguousarray(np.asarray(a, np.float32).astype(
        ml_dtypes.bfloat16))


def _f32(a):
    return np.ascontiguousarray(np.asarray(a), dtype=np.float32)


def _prep_core_inputs(stream, b_idx, half, inp):
    p = "g" if stream == 0 else "r"
    h0, h1 = half * HALF, (half + 1) * HALF
    perm = np.r_[h0:h1, 0:h0, h1:D_INNER]  # own half first

    hs = np.asarray(inp[f"{p}_hidden_states"])[b_idx]
    w_in = np.asarray(inp[f"{p}_in_proj_w"])
    w_zo = np.concatenate(
        [np.asarray(w_in[D_INNER + h0:D_INNER + h1, :]).T,
         0.5 * np.asarray(inp[f"{p}_out_w"])[:, h0:h1].T], axis=1)
    m = {
        "hid_T": _bf(hs.T),
        "w_in_x_T": _bf(w_in[:D_INNER, :][perm].T),
        "w_zo_T": _bf(w_zo),
        "cdiag_f": _bf(_diags(np.asarray(inp[f"{p}_conv_w"])[:, 0, :][perm])),
        "cdiag_b": _bf(_diags(np.asarray(inp[f"{p}_conv_w_bwd"])[:, 0, :][perm])),
        "cbias_f": _f32(np.asarray(inp[f"{p}_conv_bias"])[perm]),
        "cbias_b": _f32(np.asarray(inp[f"{p}_conv_bias_bwd"])[perm]),
        "w_x_T_f": _bf(np.asarray(inp[f"{p}_xproj_w"])[:, perm].T),
        "w_x_T_b": _bf(np.asarray(inp[f"{p}_xproj_w_bwd"])[:, perm].T),
        "w_dt_T_f": _f32(np.asarray(inp[f"{p}_dtproj_w"])[h0:h1, :].T),
        "w_dt_T_b": _f32(np.asarray(inp[f"{p}_dtproj_w_bwd"])[h0:h1, :].T),
        "dt_bias_f": _f32(np.asarray(inp[f"{p}_dtproj_bias"])[h0:h1]),
        "dt_bias_b": _f32(np.asarray(inp[f"{p}_dtproj_bias_bwd"])[h0:h1]),
        "d_f": _f32(np.asarray(inp[f"{p}_D"])[h0:h1]),
        "d_b": _f32(np.asarray(inp[f"{p}_D_bwd"])[h0:h1]),
    }
    return m


def kernel(**inputs):
    A_log = np.asarray(inputs["A_log"])
    A_log_b = np.asarray(inputs["A_log_bwd"])
    assert np.allclose(A_log, A_log[0:1, :]), "A_log must be d-independent"
    assert np.allclose(A_log_b, A_log_b[0:1, :]), "A_log_bwd must be d-independent"
    A_f = -np.exp(A_log[0].astype(np.float64))
    A_b = -np.exp(A_log_b[0].astype(np.float64))

    key = (tuple(np.round(A_f, 10)), tuple(np.round(A_b, 10)))
    if key not in _PROGRAM_CACHE:
        _PROGRAM_CACHE[key] = build_program(list(A_f), list(A_b))
    nc = _PROGRAM_CACHE[key]

    in_maps = []
    for stream in range(2):
        for b_idx in range(NBATCH):
            for half in range(2):
                in_maps.append(_prep_core_inputs(stream, b_idx, half, inputs))

    res = run_bass_kernel_spmd(nc, in_maps, list(range(8)))
    outs = [r["out"] for r in res.results]

    g_out = np.stack([outs[0] + outs[1], outs[2] + outs[3]])
    r_out = np.stack([outs[4] + outs[5], outs[6] + outs[7]])
    return (np.asarray(g_out, np.float32), np.asarray(r_out, np.float32))


# revision 30
# speedup vs baseline: 1.1953x; 1.1953x over previous
"""CoSSM (bidirectional Mamba-style SSM) Trainium2 Bass kernel.

Sharding over 8 cores: (stream g/r) x (batch 0/1) x (d_inner half 0/1).
Each core, for its (stream, batch):
  - in_proj x (all 1536 ch, channel-permuted so own half = tiles 0..5),
    recomputed per direction, fused with the causal/anticausal depthwise
    conv (diagonal PE matmuls) + native Silu, chunked over time with a
    4-column halo carried between chunks (no DRAM round trip for x)
  - xproj (full-d contraction) -> dt/B/C rows; dtproj + native Softplus
  - fwd + bwd selective scans (16 states, DVE tensor_tensor_scan);
    bwd consumes u/dl/B/C through time-reversed DMA reads (no flips)
  - D residual, bidirectional average (0.5 folded into out_proj w),
    silu(z) gate, partial out_proj -> (2048, 768)
Host sums the two partial outputs per (stream, batch).

Weights are SBUF-resident (loaded once, bf16); activations bf16 where
cheap, f32 on the delta/scan-state path.

Self-contained: hardcodes shapes. Inputs use the reference setup_inputs names.
"""
import numpy as np
import ml_dtypes

import concourse.bass as bass
import concourse.bacc as bacc
import concourse.tile as tile
import concourse.mybir as mybir
from concourse.bass_utils import run_bass_kernel_spmd

F32 = mybir.dt.float32
BF16 = mybir.dt.bfloat16
AF = mybir.ActivationFunctionType
OP = mybir.AluOpType

D_MODEL = 768
D_STATE = 16
D_CONV = 4
D_INNER = 1536
DT_RANK = 48
NBATCH = 2
L = 2048
HALF = 768
NT_FULL = 12      # 128-tiles over d_inner
NT_HALF = 6       # 128-tiles over own half
NK = 6            # 128-chunks over d_model contraction
TCH = 512         # time chunk for phase A
NTCH = L // TCH
SEG = 256         # time segment for the scan + epilogue
NSEG = L // SEG
HALO = 4

_PROGRAM_CACHE = {}


def _ap(t, offset, ap):
    return bass.AP(tensor=t.tensor, offset=offset, ap=[list(a) for a in ap])


def _bc6(view2d):
    """[128, T] AP -> [128, 6, T] read view (0-stride middle dim)."""
    a = view2d
    return bass.AP(tensor=a.tensor, offset=a.offset,
                   ap=[list(a.ap[0]), [0, NT_HALF], list(a.ap[1])])


def build_program(a_vals_f, a_vals_b):
    nc = bacc.Bacc("TRN2", target_bir_lowering=False, debug=False, num_devices=8)

    def din(name, shape, dt):
        return nc.dram_tensor(name, list(shape), dt, kind="ExternalInput").ap()

    hid_T = din("hid_T", (D_MODEL, L), BF16)
    w_in_x_T = din("w_in_x_T", (D_MODEL, D_INNER), BF16)
    w_zo_T = din("w_zo_T", (D_MODEL, 2 * D_MODEL), BF16)  # [w_in_z | 0.5*w_out]
    cdiag = {"f": din("cdiag_f", (NT_FULL * D_CONV, 128, 128), BF16),
             "b": din("cdiag_b", (NT_FULL * D_CONV, 128, 128), BF16)}
    cbias = {"f": din("cbias_f", (D_INNER,), F32),
             "b": din("cbias_b", (D_INNER,), F32)}
    w_x_T = {"f": din("w_x_T_f", (D_INNER, 80), BF16),
             "b": din("w_x_T_b", (D_INNER, 80), BF16)}
    w_dt_T = {"f": din("w_dt_T_f", (DT_RANK, HALF), F32),
              "b": din("w_dt_T_b", (DT_RANK, HALF), F32)}
    dt_bias = {"f": din("dt_bias_f", (HALF,), F32),
               "b": din("dt_bias_b", (HALF,), F32)}
    d_res = {"f": din("d_f", (HALF,), F32), "b": din("d_b", (HALF,), F32)}

    out = nc.dram_tensor("out", [L, D_MODEL], F32, kind="ExternalOutput").ap()

    u_sp = {d: nc.dram_tensor(f"u_sp_{d}", [NT_HALF, 128, L], BF16).ap()
            for d in "fb"}
    dl_sp = {d: nc.dram_tensor(f"dl_sp_{d}", [NT_HALF, 128, L], F32).ap()
             for d in "fb"}
    r_sp = {d: nc.dram_tensor(f"r_sp_{d}", [NT_HALF, 128, L], F32).ap()
            for d in "fb"}
    bc_dram = {d: nc.dram_tensor(f"bc_{d}", [2 * D_STATE, L], BF16).ap()
               for d in "fb"}

    a_vals = {"f": a_vals_f, "b": a_vals_b}

    import contextlib
    with tile.TileContext(nc) as tc, contextlib.ExitStack() as ctx:
        WPOOL = ctx.enter_context(tc.tile_pool(name="wsmall", bufs=1))
        AP_ = ctx.enter_context(tc.tile_pool(name="phaseA", bufs=1))
        SP = ctx.enter_context(tc.tile_pool(name="work", bufs=2))
        SC = ctx.enter_context(tc.tile_pool(name="scan1", bufs=1))
        SC2 = ctx.enter_context(tc.tile_pool(name="scan2", bufs=3))
        DP = ctx.enter_context(tc.tile_pool(name="phaseD", bufs=1))
        PS = ctx.enter_context(tc.tile_pool(name="psum", bufs=2, space="PSUM"))
        PS1 = ctx.enter_context(tc.tile_pool(name="psum1", bufs=1, space="PSUM"))
        PER = ctx.enter_context(tc.tile_pool(name="persist", bufs=1))

        # ---- resident weights (one DMA each) ----
        t_wbig = WPOOL.tile([128, NK, D_INNER], BF16, tag="wbig")
        nc.sync.dma_start(
            out=t_wbig,
            in_=_ap(w_in_x_T, 0,
                    [[D_INNER, 128], [128 * D_INNER, NK], [1, D_INNER]]))

        def load_cols(src, n, tagn):
            t = WPOOL.tile([128, n], F32, tag=tagn)
            nc.sync.dma_start(out=t, in_=_ap(src, 0, [[1, 128], [128, n]]))
            return t

        t_cbias = {d: load_cols(cbias[d], NT_FULL, f"cb{d}") for d in "fb"}
        t_dtb = {d: load_cols(dt_bias[d], NT_HALF, f"db{d}") for d in "fb"}
        t_dcol = {d: load_cols(d_res[d], NT_HALF, f"dd{d}") for d in "fb"}
        t_wx = {}
        for d in "fb":
            t = WPOOL.tile([128, NT_FULL, 80], BF16, tag=f"wx{d}")
            nc.sync.dma_start(
                out=t, in_=_ap(w_x_T[d], 0,
                               [[80, 128], [128 * 80, NT_FULL], [1, 80]]))
            t_wx[d] = t
        t_wdt = {}
        for d in "fb":
            t = WPOOL.tile([128, HALF], F32, tag=f"wdt{d}")
            nc.sync.dma_start(out=t[0:DT_RANK, :], in_=w_dt_T[d])
            t_wdt[d] = t

        # conv diag tiles: loaded per direction into one aliased slot
        def load_cdiag(d):
            t = WPOOL.tile([128, NT_FULL * D_CONV, 128], BF16, tag="cdiag")
            nc.sync.dma_start(
                out=t, in_=_ap(cdiag[d], 0,
                               [[128, 128], [128 * 128, NT_FULL * D_CONV],
                                [1, 128]]))
            return t

        t_xdbl = PER.tile([128, L], F32, tag="xdbl")
        t_xw = PER.tile([128, NT_FULL, TCH + HALO], BF16, tag="xw")
        carry = {d: PER.tile([128, D_STATE, NT_HALF], F32, tag=f"carry{d}",
                             name=f"carry_{d}")
                 for d in "fb"}

        # ---- phase A: fused in_proj + conv + silu + xproj + dt ----
        def emit_phaseA_chunk(d, tci, t_cd):
            t0 = tci * TCH
            first = (tci == 0) if d == "f" else (tci == NTCH - 1)
            th = AP_.tile([128, NK, TCH], BF16, tag="hidw", bufs=2)
            nc.sync.dma_start(
                out=th, in_=_ap(hid_T, t0, [[L, 128], [128 * L, NK], [1, TCH]]))
            # halo carry in xw: fwd cols 0:4 <- prev cols 512:516 (or zero);
            # bwd cols 512:516 <- prev cols 0:4 (or zero)
            if d == "f":
                if first:
                    nc.vector.memset(t_xw[:, :, 0:HALO].bitcast(F32), 0.0)
                else:
                    nc.vector.tensor_copy(t_xw[:, :, 0:HALO],
                                          t_xw[:, :, TCH:TCH + HALO])
            else:
                if first:
                    nc.vector.memset(t_xw[:, :, TCH:].bitcast(F32), 0.0)
                else:
                    nc.vector.tensor_copy(t_xw[:, :, TCH:TCH + HALO],
                                          t_xw[:, :, 0:HALO])
            xcol = HALO if d == "f" else 0

            t_useg = AP_.tile([128, NT_HALF, TCH], BF16, tag="useg")
            t_dlseg = AP_.tile([128, NT_HALF, TCH], F32, tag="dlseg")
            px = PS1.tile([128, TCH], F32, tag="xproj")
            for i in range(NT_FULL):
                ip = PS.tile([128, TCH], F32, tag="mm512")
                for k in range(NK):
                    nc.tensor.matmul(ip[:], t_wbig[:, k, i * 128:(i + 1) * 128],
                                     th[:, k, :],
                                     start=(k == 0), stop=(k == NK - 1))
                nc.scalar.copy(t_xw[:, i, xcol:xcol + TCH], ip[:])
                cp = PS.tile([128, TCH], F32, tag="mm512")
                for k in range(D_CONV):
                    off = 1 + k if d == "f" else 3 - k
                    nc.tensor.matmul(cp[:], t_cd[:, i * D_CONV + k, :],
                                     t_xw[:, i, off:off + TCH],
                                     start=(k == 0), stop=(k == D_CONV - 1))
                if i < NT_HALF:
                    u_i = t_useg[:, i, :]
                else:
                    u_hi = SP.tile([128, TCH], BF16, tag="u_hi")
                    u_i = u_hi[:]
                nc.scalar.activation(u_i, cp[:], AF.Silu,
                                     bias=t_cbias[d][:, i:i + 1], scale=1.0)
                nc.tensor.matmul(px[0:80, :], t_wx[d][:, i, :], u_i,
                                 start=(i == 0), stop=(i == NT_FULL - 1))
            nc.sync.dma_start(
                out=_ap(u_sp[d], t0, [[L, 128], [128 * L, NT_HALF], [1, TCH]]),
                in_=t_useg[:])
            nc.scalar.copy(t_xdbl[0:80, t0:t0 + TCH], px[0:80, :])
            for m in range(NT_HALF):
                dp = PS.tile([128, TCH], F32, tag="mm512")
                nc.tensor.matmul(dp[:],
                                 t_wdt[d][0:DT_RANK, m * 128:(m + 1) * 128],
                                 t_xdbl[0:DT_RANK, t0:t0 + TCH],
                                 start=True, stop=True)
                nc.scalar.activation(t_dlseg[:, m, :], dp[:], AF.Exp,
                                     bias=t_dtb[d][:, m:m + 1], scale=1.0)
            # softplus: one batched in-place ln(1 + e^x) pass (avoids
            # per-m exp<->ln activation-table thrash)
            nc.scalar.activation(t_dlseg[:], t_dlseg[:], AF.Ln,
                                 bias=1.0, scale=1.0)
            nc.sync.dma_start(
                out=_ap(dl_sp[d], t0, [[L, 128], [128 * L, NT_HALF], [1, TCH]]),
                in_=t_dlseg[:])

        # ---- scan ----
        def emit_scan_seg(d, s):
            # Scan seg s covers natural times [base, base+SEG). For d == "b"
            # all data stays in natural time order; only the scan instruction
            # operands use reversed views so the recurrence runs backward.
            rev = (d == "b")
            base = (L - (s + 1) * SEG) if rev else s * SEG
            u_seg = SC.tile([128, NT_HALF, SEG], BF16, tag="u_seg", bufs=2)
            nc.sync.dma_start(
                out=u_seg,
                in_=_ap(u_sp[d], base,
                        [[L, 128], [128 * L, NT_HALF], [1, SEG]]))
            dl_seg = SC.tile([128, NT_HALF, SEG], F32, tag="dl_seg")
            nc.sync.dma_start(
                out=dl_seg,
                in_=_ap(dl_sp[d], base,
                        [[L, 128], [128 * L, NT_HALF], [1, SEG]]))
            bc = SC.tile([128, 2 * D_STATE, SEG], BF16, tag="bc")
            nc.sync.dma_start(
                out=bc,
                in_=_ap(bc_dram[d], base,
                        [[0, 128], [L, 2 * D_STATE], [1, SEG]]))
            ud_seg = SC.tile([128, NT_HALF, SEG], BF16, tag="ud_seg")
            nc.vector.tensor_tensor(ud_seg[:], dl_seg[:], u_seg[:], OP.mult)
            y_seg = SC.tile([128, NT_HALF, SEG], F32, tag="y_seg")
            for j in range(D_STATE):
                a_j = SC2.tile([128, NT_HALF, SEG], BF16, tag="a_j")
                nc.scalar.activation(a_j[:], dl_seg[:], AF.Exp, bias=0.0,
                                     scale=float(a_vals[d][j]))
                b_j = SC2.tile([128, NT_HALF, SEG], BF16, tag="b_j")
                nc.vector.tensor_tensor(b_j[:], ud_seg[:], _bc6(bc[:, j, :]),
                                        OP.mult)
                h_j = SC2.tile([128, NT_HALF, SEG], BF16, tag="h_j")

                def sop(t, i):
                    v = t[:, i, :]
                    if not rev:
                        return v
                    return bass.AP(tensor=v.tensor, offset=v.offset + SEG - 1,
                                   ap=[list(v.ap[0]), [-1, SEG]])

                for i in range(NT_HALF):
                    init = 0.0 if s == 0 else carry[d][:, j, i:i + 1]
                    nc.vector.tensor_tensor_scan(
                        sop(h_j, i), sop(a_j, i), sop(b_j, i), init,
                        OP.mult, OP.add)
                if s < NSEG - 1:
                    # scan-order last element: natural col SEG-1 (fwd), 0 (bwd)
                    nc.vector.tensor_copy(
                        carry[d][:, j, :],
                        bass.AP(tensor=h_j.tensor,
                                offset=h_j[:].offset +
                                (0 if rev else SEG - 1),
                                ap=[list(h_j[:].ap[0]), [SEG, NT_HALF]]))
                nc.vector.tensor_tensor(h_j[:], h_j[:],
                                        _bc6(bc[:, D_STATE + j, :]), OP.mult)
                if j == 0:
                    nc.gpsimd.tensor_copy(out=y_seg[:], in_=h_j[:])
                elif j % 4 == 1:
                    nc.vector.tensor_tensor(y_seg[:], y_seg[:], h_j[:], OP.add)
                else:
                    nc.gpsimd.tensor_tensor(y_seg[:], y_seg[:], h_j[:], OP.add)
            r_seg = SC.tile([128, NT_HALF, SEG], F32, tag="r_seg")
            for i in range(NT_HALF):
                nc.vector.scalar_tensor_tensor(
                    r_seg[:, i, :], u_seg[:, i, :],
                    t_dcol[d][:, i:i + 1], y_seg[:, i, :], OP.mult, OP.add)
            nc.sync.dma_start(
                out=_ap(r_sp[d], base,
                        [[L, 128], [128 * L, NT_HALF], [1, SEG]]),
                in_=r_seg[:])

        # ---- phase D: z-gate + combine + out_proj ----
        def emit_D_seg(s):
            t0 = s * SEG
            th = AP_.tile([128, NK, SEG], BF16, tag="hidw", bufs=2)
            nc.sync.dma_start(
                out=th, in_=_ap(hid_T, t0, [[L, 128], [128 * L, NK], [1, SEG]]))
            gate = DP.tile([128, NT_HALF, SEG], BF16, tag="gate")
            for m in range(NT_HALF):
                zp = PS.tile([128, SEG], F32, tag="mm512")
                for k in range(NK):
                    nc.tensor.matmul(zp[:], t_wbig[:, k, m * 128:(m + 1) * 128],
                                     th[:, k, :],
                                     start=(k == 0), stop=(k == NK - 1))
                nc.scalar.activation(gate[:, m, :], zp[:], AF.Silu, bias=0.0,
                                     scale=1.0)
            rf = DP.tile([128, NT_HALF, SEG], F32, tag="rf")
            nc.sync.dma_start(
                out=rf, in_=_ap(r_sp["f"], t0,
                                [[L, 128], [128 * L, NT_HALF], [1, SEG]]))
            rb = DP.tile([128, NT_HALF, SEG], F32, tag="rb")
            nc.sync.dma_start(
                out=rb, in_=_ap(r_sp["b"], t0,
                                [[L, 128], [128 * L, NT_HALF], [1, SEG]]))
            nc.vector.tensor_tensor(rf[:], rf[:], rb[:], OP.add)
            yg = DP.tile([128, NT_HALF, SEG], BF16, tag="yg")
            nc.vector.tensor_tensor(yg[:], rf[:], gate[:], OP.mult)
            for tcc in range(SEG // 128):
                oseg = SP.tile([128, D_MODEL], F32, tag="oseg")
                for nh in range(2):
                    po = PS.tile([128, 384], F32, tag="oproj")
                    for i in range(NT_HALF):
                        nc.tensor.matmul(
                            po[:], yg[:, i, tcc * 128:(tcc + 1) * 128],
                            t_wbig[:, i, D_MODEL + nh * 384:
                                   D_MODEL + (nh + 1) * 384],
                            start=(i == 0), stop=(i == NT_HALF - 1))
                    nc.scalar.copy(oseg[:, nh * 384:(nh + 1) * 384], po[:])
                nc.sync.dma_start(
                    out=out[t0 + tcc * 128:t0 + (tcc + 1) * 128, :],
                    in_=oseg[:])

        # ---- sequencing: overlap scan(f) with A(b), scan(b) with D ----
        t_cd = load_cdiag("f")
        for tci in range(NTCH):
            emit_phaseA_chunk("f", tci, t_cd)
        nc.gpsimd.dma_start(out=bc_dram["f"],
                            in_=t_xdbl[DT_RANK:DT_RANK + 2 * D_STATE, :])
        t_cd = load_cdiag("b")
        for s in range(NSEG):
            emit_scan_seg("f", s)
            if s % 2 == 1:
                emit_phaseA_chunk("b", NTCH - 1 - s // 2, t_cd)
        nc.gpsimd.dma_start(out=bc_dram["b"],
                            in_=t_xdbl[DT_RANK:DT_RANK + 2 * D_STATE, :])
        # reload wbig slot with [w_in_z | 0.5*w_out] for phase D
        nc.sync.dma_start(
            out=t_wbig,
            in_=_ap(w_zo_T, 0,
                    [[2 * D_MODEL, 128], [128 * 2 * D_MODEL, NK],
                     [1, 2 * D_MODEL]]))
        for s in range(NSEG):
            emit_scan_seg("b", s)
            emit_D_seg(NSEG - 1 - s)

    nc.compile()
    return nc


def _diags(w):  # (1536, 4) -> (48, 128, 128) diag tiles
    o = np.zeros((NT_FULL * D_CONV, 128, 128), np.float32)
    for i in range(NT_FULL):
        for k in range(D_CONV):
            np.fill_diagonal(o[i * D_CONV + k], w[i * 128:(i + 1) * 128, k])
    return o


def _bf(a):
    return np.ascontiguousarray(np.asarray(a, np.float32).astype(
        ml_dtypes.bfloat16))


def _f32(a):
    return np.ascontiguousarray(np.asarray(a), dtype=np.float32)


def _prep_core_inputs(stream, b_idx, half, inp):
    p = "g" if stream == 0 else "r"
    h0, h1 = half * HALF, (half + 1) * HALF
    perm = np.r_[h0:h1, 0:h0, h1:D_INNER]  # own half first

    hs = np.asarray(inp[f"{p}_hidden_states"])[b_idx]
    w_in = np.asarray(inp[f"{p}_in_proj_w"])
    w_zo = np.concatenate(
        [np.asarray(w_in[D_INNER + h0:D_INNER + h1, :]).T,
         0.5 * np.asarray(inp[f"{p}_out_w"])[:, h0:h1].T], axis=1)
    m = {
        "hid_T": _bf(hs.T),
        "w_in_x_T": _bf(w_in[:D_INNER, :][perm].T),
        "w_zo_T": _bf(w_zo),
        "cdiag_f": _bf(_diags(np.asarray(inp[f"{p}_conv_w"])[:, 0, :][perm])),
        "cdiag_b": _bf(_diags(np.asarray(inp[f"{p}_conv_w_bwd"])[:, 0, :][perm])),
        "cbias_f": _f32(np.asarray(inp[f"{p}_conv_bias"])[perm]),
        "cbias_b": _f32(np.asarray(inp[f"{p}_conv_bias_bwd"])[perm]),
        "w_x_T_f": _bf(np.asarray(inp[f"{p}_xproj_w"])[:, perm].T),
        "w_x_T_b": _bf(np.asarray(inp[f"{p}_xproj_w_bwd"])[:, perm].T),
        "w_dt_T_f": _f32(np.asarray(inp[f"{p}_dtproj_w"])[h0:h1, :].T),
        "w_dt_T_b": _f32(np.asarray(inp[f"{p}_dtproj_w_bwd"])[h0:h1, :].T),
        "dt_bias_f": _f32(np.asarray(inp[f"{p}_dtproj_bias"])[h0:h1]),
        "dt_bias_b": _f32(np.asarray(inp[f"{p}_dtproj_bias_bwd"])[h0:h1]),
        "d_f": _f32(np.asarray(inp[f"{p}_D"])[h0:h1]),
        "d_b": _f32(np.asarray(inp[f"{p}_D_bwd"])[h0:h1]),
    }
    return m


def kernel(**inputs):
    A_log = np.asarray(inputs["A_log"])
    A_log_b = np.asarray(inputs["A_log_bwd"])
    assert np.allclose(A_log, A_log[0:1, :]), "A_log must be d-independent"
    assert np.allclose(A_log_b, A_log_b[0:1, :]), "A_log_bwd must be d-independent"
    A_f = -np.exp(A_log[0].astype(np.float64))
    A_b = -np.exp(A_log_b[0].astype(np.float64))

    key = (tuple(np.round(A_f, 10)), tuple(np.round(A_b, 10)))
    if key not in _PROGRAM_CACHE:
        _PROGRAM_CACHE[key] = build_program(list(A_f), list(A_b))
    nc = _PROGRAM_CACHE[key]

    in_maps = []
    for stream in range(2):
        for b_idx in range(NBATCH):
            for half in range(2):
                in_maps.append(_prep_core_inputs(stream, b_idx, half, inputs))

    res = run_bass_kernel_spmd(nc, in_maps, list(range(8)))
    outs = [r["out"] for r in res.results]

    g_out = np.stack([outs[0] + outs[1], outs[2] + outs[3]])
    r_out = np.stack([outs[4] + outs[5], outs[6] + outs[7]])
    return (np.asarray(g_out, np.float32), np.asarray(r_out, np.float32))
